# revision 42
# baseline (speedup 1.0000x reference)
"""LSTM autoencoder (B=8192, T=50, F=24; H1=64, LAT=32, H3=64) on 8 trn2 cores.

v15: packed-halves, fold-free layout. The partition dim carries
feature x batch-half (64 feats x 2 halves = 128 partitions, FD=512
columns) so every elementwise op runs full width and no cross-partition
fold copies are needed. Each LSTM1 gate gets its own PSUM bank filled
by two accumulating matmuls: a packed x-side MM and a block-diagonal
h-side MM; relu(g) folds into the scalar_tensor_tensor that forms
i*relu(g) (sigmoid/relu elision: c >= 0 so h = o*relu(c) = o*c).
LSTM2 interleaves with LSTM1 at matmul-lag 2 / elementwise-lag 3 (its
h-side MMs are emitted after the elementwise block that writes h2 --
emitting them earlier reads a stale H2). Phase B (decoder LSTM3 +
dense) runs two independent FD/2 column streams to hide the recurrence
chain. All weights ship in one packed [128, 21*128] DMA. The dense
bias is applied host-side (it is outside all nonlinearities).
LSTM2's elementwise emission order (SFI2, IG2, FC2, C2add before SO2,
H2mul) matters: the LSTM2 h2-loop is the period-setter in phase A and
this ordering minimizes its schedule gaps.
Measured: 346 us HW (baseline v7: 539 us same-day), rel_err 3.9e-3.
"""

import os
import sys

import numpy as np

sys.path.insert(0, "/opt/trn_rl_repo")

import concourse.bass as bass
import concourse.mybir as mybir
from concourse.tile import TileContext
from contextlib import ExitStack

B, T, F = 8192, 50, 24
H1, LAT, H3 = 64, 32, 64
NCORES = 8
Bc = B // NCORES  # 1024
FD = Bc // 2  # 512 free-dim columns per packed op

f16 = mybir.dt.float16
f32 = mybir.dt.float32
AF = mybir.ActivationFunctionType
Alu = mybir.AluOpType

_CACHE = {}

XROWS = 96  # x tile rows: [x_h0(0:24); 1(24); pad; x_h1(64:88); 1(88); pad]

# ---------------------------------------------------------------------------
# Toolchain compat: the walrus build in this container predates two features
# the current Tile framework emits: replace the EVSEM tail barrier with the
# legacy pseudo barrier, and split >1 sem waits per instruction into
# single-wait NoOps.
# ---------------------------------------------------------------------------

bass.Bass.all_engine_barrier = (
    lambda self, *, sem_only=False: self._nrt_pseudo_barrier()
)
bass.Bass.multi_engine_barrier = lambda self, engines: self._nrt_pseudo_barrier()


def _split_multi_waits(js: bytes) -> bytes:
    import json

    m = json.loads(js)
    for fn in m["functions"]:
        for blk in fn["blocks"]:
            out = []
            for inst in blk["instructions"]:
                si = inst.get("sync_info")
                waits = (si or {}).get("on_wait") or []
                if len(waits) > 1:
                    for k, w in enumerate(waits[:-1]):
                        out.append(
                            {
                                "name": f"{inst['name']}_w{k}",
                                "engine": inst["engine"],
                                "opcode": "NoOp",
                                "debug": inst.get("debug", 0),
                                "ins": [],
                                "outs": [],
                                "sync_info": {"on_update": [], "on_wait": [w]},
                            }
                        )
                    si["on_wait"] = [waits[-1]]
                out.append(inst)
            blk["instructions"] = out
    return json.dumps(m).encode()


def _wrap_to_json(nc):
    orig = nc.to_json_bytes
    nc.to_json_bytes = lambda: _split_multi_waits(orig())
    return nc


def _build_nc(repeat=1):
    nc = bass.Bass()

    xT_d = nc.dram_tensor("xT", [T, XROWS, FD], f16, kind="ExternalInput")
    # All weights packed into one [128, 21*128] f16 tensor (one DMA):
    # col blocks: wx1[4] | uh1[4] | wx2[2] | uh2[2] | wz3[4] | uh3[4] | wd
    WCOLS = 21 * 128
    wall_d = nc.dram_tensor("wall", [128, WCOLS], f16, kind="ExternalInput")
    yT_d = nc.dram_tensor("yT", [T, 64, FD], f16, kind="ExternalOutput")

    with TileContext(nc) as tc:
     for _rep in range(repeat):
      with ExitStack() as ctx:
        wp = ctx.enter_context(tc.tile_pool(name=f"wp{_rep}", bufs=1))
        st = ctx.enter_context(tc.tile_pool(name=f"st{_rep}", bufs=1))
        sp = ctx.enter_context(tc.tile_pool(name=f"sp{_rep}", bufs=2))
        op = ctx.enter_context(tc.tile_pool(name=f"op{_rep}", bufs=3))

        wall = wp.tile([128, WCOLS], f16, name="wall")
        nc.sync.dma_start(out=wall, in_=wall_d[:])
        def wslice(i, rows):
            return wall[0:rows, 128 * i : 128 * i + 128]
        wx1 = [wslice(g, XROWS) for g in range(4)]
        uh1 = [wslice(4 + g, 128) for g in range(4)]
        wx2 = [wslice(8 + g, 128) for g in range(2)]
        uh2 = [wslice(10 + g, 65) for g in range(2)]
        wz3 = [wslice(12 + g, 65) for g in range(4)]
        uh3 = [wslice(16 + g, 128) for g in range(4)]
        wd = wall[0:128, 20 * 128 : 20 * 128 + 64]

        # ---- state ---------------------------------------------------------
        xr = [st.tile([XROWS, FD], f16, name=f"X{i}") for i in range(4)]
        h1r = [st.tile([128, FD], f16, name=f"H1_{i}") for i in range(4)]
        C1 = st.tile([128, FD], f16, name="C1")
        H2 = st.tile([65, FD], f16, name="H2")  # row 64 = 1.0 (bias row)
        C2 = st.tile([64, FD], f16, name="C2")

        nc.vector.memset(h1r[3], 0)  # h1_{-1}
        nc.vector.memset(C1, 0)
        nc.vector.memset(H2[0:64, :], 0)
        nc.vector.memset(H2[64:65, :], 1.0)
        nc.vector.memset(C2, 0)

        nc.sync.dma_start(out=xr[0], in_=xT_d[0])
        nc.sync.dma_start(out=xr[1], in_=xT_d[1])

        # ---- phase A: LSTM1 (t=k) + lagged LSTM2 ---------------------------
        # Weight-array gate order: 0=f, 1=i, 2=g, 3=o.
        # LSTM2's matmuls lag 2 steps (t2m) but its sigmoid+elementwise lag 3
        # (t2e): the elementwise block then sits AFTER the current LSTM1
        # chain in every engine's priority order, so the scheduler can never
        # pop an LSTM2 tail op ahead of a chain op, and SL2's ready time
        # falls into ACT's idle window after SFO instead of racing SI.
        # psL2 is double-buffered (a/b) so the two in-flight LSTM2 steps
        # don't serialize on the bank.
        with tc.tile_pool(name=f"ppA{_rep}", bufs=1, space="PSUM") as ppa:
          psL2s = {}
          for k in range(T + 3):
            if k + 2 < T:
                nc.sync.dma_start(out=xr[(k + 2) % 4], in_=xT_d[k + 2])

            t2m = k - 2  # LSTM2 matmul step
            t2 = k - 3  # LSTM2 elementwise step
            if k < T:
                X = xr[k % 4]
                Hp = h1r[(k + 3) % 4]  # h1_{k-1}
                psI = ppa.tile([128, FD], f32, tag="psI")
                psFO = ppa.tile([128, 2 * FD], f32, tag="psFO")
                psG = ppa.tile([128, FD], f32, tag="psG")
                # x-MMs for I/F/O run during the pre-h-MM stall; the three
                # chain-critical h-MMs follow immediately after h1_{k-1}
                # lands; G (consumed later, by IG1) and LSTM2 trail.
                nc.tensor.matmul(psI, wx1[1], X, start=True, stop=False)
                nc.tensor.matmul(
                    psFO[:, 0:FD], wx1[0], X, start=True, stop=False
                )
                nc.tensor.matmul(
                    psFO[:, FD : 2 * FD], wx1[3], X, start=True, stop=False
                )
                nc.tensor.matmul(psI, uh1[1], Hp, start=False, stop=True)
                nc.tensor.matmul(psFO[:, 0:FD], uh1[0], Hp, start=False, stop=True)
                nc.tensor.matmul(
                    psFO[:, FD : 2 * FD], uh1[3], Hp, start=False, stop=True
                )
                nc.tensor.matmul(psG, wx1[2], X, start=True, stop=False)
                nc.tensor.matmul(psG, uh1[2], Hp, start=False, stop=True)

            if 0 <= t2m < T:
                psL2 = ppa.tile(
                    [128, 2 * FD], f32, tag=f"psL2{t2m % 2}", name=f"psL2{t2m % 2}"
                )
                psL2s[t2m] = psL2
                nc.tensor.matmul(
                    psL2[:, 0:FD], wx2[0], h1r[t2m % 4], start=True, stop=False
                )
                nc.tensor.matmul(
                    psL2[:, FD : 2 * FD], wx2[1], h1r[t2m % 4],
                    start=True, stop=False,
                )

            if k < T:
                SI = sp.tile([128, FD], f16, tag="SI")
                nc.scalar.activation(SI, psI, AF.Sigmoid)
                SIg = SI
                SF = sp.tile([128, FD], f16, tag="SF")
                nc.scalar.activation(SF, psFO[:, 0:FD], AF.Sigmoid)
                SO = sp.tile([128, FD], f16, tag="SO")
                nc.scalar.activation(SO, psFO[:, FD : 2 * FD], AF.Sigmoid)
                SOg = SO

                IG1 = sp.tile([128, FD], f16, tag="IG1")
                nc.vector.scalar_tensor_tensor(
                    IG1, psG, 0.0, SI, Alu.max, Alu.mult
                )  # relu(g1) * si1
                FC1 = sp.tile([128, FD], f16, tag="FC1")
                nc.vector.tensor_mul(FC1, SF, C1)
                nc.vector.tensor_add(C1, FC1, IG1)
                nc.vector.tensor_mul(h1r[k % 4], SO, C1)

            if 0 <= t2 < T:
                psL2 = psL2s.pop(t2)
                SFI2 = sp.tile([128, FD], f16, tag="SFI2")
                nc.scalar.activation(SFI2, psL2[:, 0:FD], AF.Sigmoid)
                FC2 = sp.tile([64, FD], f16, tag="FC2")
                nc.vector.tensor_mul(FC2, SFI2[0:64, :], C2)
                IG2 = sp.tile([64, FD], f16, tag="IG2")
                nc.vector.scalar_tensor_tensor(
                    IG2, psL2[64:128, FD : 2 * FD], 0.0, SFI2[64:128, :],
                    Alu.max, Alu.mult,
                )  # relu(g2) * si2
                nc.vector.tensor_add(C2, FC2, IG2)
                SO2 = sp.tile([64, FD], f16, tag="SO2")
                nc.scalar.activation(SO2, psL2[0:64, FD : 2 * FD], AF.Sigmoid)
                nc.vector.tensor_mul(H2[0:64, :], SO2, C2)

            if 0 <= t2m < T:
                # h-side MMs AFTER the elementwise block: they contract
                # h2_{t2m-1}, which the H2mul just above (step t2 = t2m-1)
                # writes. Emitting them earlier would read a stale H2.
                psL2 = psL2s[t2m]
                nc.tensor.matmul(psL2[:, 0:FD], uh2[0], H2, start=False, stop=True)
                nc.tensor.matmul(
                    psL2[:, FD : 2 * FD], uh2[1], H2, start=False, stop=True
                )

        # ---- phase B: LSTM3 + dense, two FD/2 column streams ---------------
        HF = FD // 2  # 256
        css = (slice(0, HF), slice(HF, FD))
        H3s = [st.tile([128, HF], f16, name=f"H3_{s}") for s in range(2)]
        C3s = [st.tile([128, HF], f16, name=f"C3_{s}") for s in range(2)]
        for s in range(2):
            nc.vector.memset(H3s[s], 0)
            nc.vector.memset(C3s[s], 0)

        with tc.tile_pool(name=f"ppB{_rep}", bufs=1, space="PSUM") as ppb:
          for t in range(T):
            psI3 = [None, None]
            psFO3 = [None, None]
            psG3 = [None, None]
            psD3 = [None, None]
            # z-side MMs (only need H2 + a free bank) lead the PE FIFO
            for s in range(2):
                psI3[s] = ppb.tile([128, HF], f32, tag=f"psI3{s}", name=f"psI3{s}")
                psFO3[s] = ppb.tile(
                    [128, 2 * HF], f32, tag=f"psFO3{s}", name=f"psFO3{s}"
                )
                psG3[s] = ppb.tile([128, HF], f32, tag=f"psG3{s}", name=f"psG3{s}")
                Z = H2[:, css[s]]
                nc.tensor.matmul(psI3[s], wz3[1][:, :], Z, start=True, stop=False)
                nc.tensor.matmul(
                    psFO3[s][:, 0:HF], wz3[0][:, :], Z, start=True, stop=False
                )
                nc.tensor.matmul(
                    psFO3[s][:, HF : 2 * HF], wz3[3][:, :], Z,
                    start=True, stop=False,
                )
                nc.tensor.matmul(psG3[s], wz3[2][:, :], Z, start=True, stop=False)
            for s in range(2):
                nc.tensor.matmul(psI3[s], uh3[1], H3s[s], start=False, stop=True)
                nc.tensor.matmul(
                    psFO3[s][:, 0:HF], uh3[0], H3s[s], start=False, stop=True
                )
                nc.tensor.matmul(
                    psFO3[s][:, HF : 2 * HF], uh3[3], H3s[s],
                    start=False, stop=True,
                )
                nc.tensor.matmul(psG3[s], uh3[2], H3s[s], start=False, stop=True)

            SI3s, SFO3s = [], []
            for s in range(2):
                SI3 = sp.tile([128, HF], f16, tag=f"SI3{s}", name=f"SI3{s}")
                nc.scalar.activation(SI3, psI3[s], AF.Sigmoid)
                SFO3 = sp.tile(
                    [128, 2 * HF], f16, tag=f"SFO3{s}", name=f"SFO3{s}"
                )
                nc.scalar.activation(SFO3, psFO3[s], AF.Sigmoid)
                SI3s.append(SI3)
                SFO3s.append(SFO3)

            for s in range(2):
                IG3 = sp.tile([128, HF], f16, tag=f"IG3{s}", name=f"IG3{s}")
                nc.vector.scalar_tensor_tensor(
                    IG3, psG3[s], 0.0, SI3s[s], Alu.max, Alu.mult
                )
                FC3 = sp.tile([128, HF], f16, tag=f"FC3{s}", name=f"FC3{s}")
                nc.vector.tensor_mul(FC3, SFO3s[s][:, 0:HF], C3s[s])
                nc.vector.tensor_add(C3s[s], FC3, IG3)
                nc.vector.tensor_mul(
                    H3s[s], SFO3s[s][:, HF : 2 * HF], C3s[s]
                )  # h3_t

            for s in range(2):
                psD3[s] = ppb.tile([64, HF], f32, tag=f"psD{s}", name=f"psD{s}")
                nc.tensor.matmul(psD3[s], wd, H3s[s], start=True, stop=True)
            for s in range(2):
                yt = op.tile([64, HF], f16, tag=f"yt{s}", name=f"yt{s}")
                nc.vector.tensor_copy(yt, psD3[s])  # bd added host-side
                nc.sync.dma_start(out=yT_d[t][:, css[s]], in_=yt)

    return nc


def _prep_inputs(inputs):
    """Host-side: shard batch, pack weights/x into the packed-halves layout."""
    x = np.asarray(inputs["x"], np.float32)
    W1, U1, b1 = (np.asarray(inputs[k], np.float32) for k in ("W1", "U1", "b1"))
    W2, U2, b2 = (np.asarray(inputs[k], np.float32) for k in ("W2", "U2", "b2"))
    W3, U3, b3 = (np.asarray(inputs[k], np.float32) for k in ("W3", "U3", "b3"))
    Wd, bd = (np.asarray(inputs[k], np.float32) for k in ("Wd", "bd"))

    # Reference gate column order is (i, f, g, o), each H wide.
    def gcols(Wm, H, g):
        idx = {"i": 0, "f": 1, "g": 2, "o": 3}[g]
        return Wm[:, idx * H : (idx + 1) * H]

    GATES = ("f", "i", "g", "o")

    # LSTM1: x-side [XROWS,128] (rows 0:24 x_h0, 24 bias, 64:88 x_h1, 88 bias)
    # h-side blockdiag [128,128]
    wx1 = np.zeros((4, XROWS, 128), np.float32)
    uh1 = np.zeros((4, 128, 128), np.float32)
    for gi, g in enumerate(GATES):
        Wg, Ug, bg = gcols(W1, H1, g), gcols(U1, H1, g), gcols(b1[None], H1, g)[0]
        wx1[gi, 0:24, 0:64] = Wg
        wx1[gi, 24, 0:64] = bg
        wx1[gi, 64:88, 64:128] = Wg
        wx1[gi, 88, 64:128] = bg
        uh1[gi, 0:64, 0:64] = Ug
        uh1[gi, 64:128, 64:128] = Ug

    # LSTM2 gate pairs (f,i) and (o,g): bank cols = [gA_h0, gA_h1, gB_h0, gB_h1]
    wx2 = np.zeros((2, 128, 128), np.float32)
    uh2 = np.zeros((2, 65, 128), np.float32)
    for pi, (ga, gb) in enumerate((("f", "i"), ("o", "g"))):
        for half in range(2):
            r0, r1 = half * 64, half * 64 + 64  # h1 rows for this half
            s0, s1 = half * 32, half * 32 + 32  # h2 rows for this half
            wx2[pi, r0:r1, s0 : s0 + 32] = gcols(W2, LAT, ga)
            wx2[pi, r0:r1, 64 + s0 : 64 + s0 + 32] = gcols(W2, LAT, gb)
            uh2[pi, s0:s1, s0 : s0 + 32] = gcols(U2, LAT, ga)
            uh2[pi, s0:s1, 64 + s0 : 64 + s0 + 32] = gcols(U2, LAT, gb)
            uh2[pi, 64, s0 : s0 + 32] = gcols(b2[None], LAT, ga)[0]
            uh2[pi, 64, 64 + s0 : 64 + s0 + 32] = gcols(b2[None], LAT, gb)[0]

    # LSTM3: z-side [65,128] (z rows 0:32 h0, 32:64 h1, 64 bias), h-side [128,128]
    wz3 = np.zeros((4, 65, 128), np.float32)
    uh3 = np.zeros((4, 128, 128), np.float32)
    for gi, g in enumerate(GATES):
        Wg, Ug, bg = gcols(W3, H3, g), gcols(U3, H3, g), gcols(b3[None], H3, g)[0]
        wz3[gi, 0:32, 0:64] = Wg
        wz3[gi, 32:64, 64:128] = Wg
        wz3[gi, 64, 0:64] = bg
        wz3[gi, 64, 64:128] = bg
        uh3[gi, 0:64, 0:64] = Ug
        uh3[gi, 64:128, 64:128] = Ug

    wd = np.zeros((128, 64), np.float32)
    wd[0:64, 0:24] = Wd
    wd[64:128, 32:56] = Wd
    wall = np.zeros((128, 21 * 128), np.float32)
    for g in range(4):
        wall[0:XROWS, 128 * g : 128 * g + 128] = wx1[g]
        wall[:, 128 * (4 + g) : 128 * (4 + g) + 128] = uh1[g]
        wall[0:65, 128 * (12 + g) : 128 * (12 + g) + 128] = wz3[g]
        wall[:, 128 * (16 + g) : 128 * (16 + g) + 128] = uh3[g]
    for g in range(2):
        wall[:, 128 * (8 + g) : 128 * (8 + g) + 128] = wx2[g]
        wall[0:65, 128 * (10 + g) : 128 * (10 + g) + 128] = uh2[g]
    wall[:, 20 * 128 : 20 * 128 + 64] = wd
    w16 = {"wall": wall.astype(np.float16)}

    in_maps = []
    for c in range(NCORES):
        xc = x[c * Bc : (c + 1) * Bc]  # [Bc, T, F]
        xt = np.zeros((T, XROWS, FD), np.float16)
        xt[:, 0:24, :] = xc[0:FD].transpose(1, 2, 0)
        xt[:, 24, :] = 1.0
        xt[:, 64:88, :] = xc[FD:Bc].transpose(1, 2, 0)
        xt[:, 88, :] = 1.0
        m = {"xT": np.ascontiguousarray(xt)}
        m.update(w16)
        in_maps.append(m)
    return in_maps


def _make_runner(nc):
    """Compile nc once into a sharded 8-core jit; returns run(in_maps)->results."""
    import jax
    from jax.sharding import Mesh, PartitionSpec
    from jax.experimental.shard_map import shard_map
    from concourse import bass2jax, mybir as _mb

    bass2jax.install_neuronx_cc_hook()

    partition_name = nc.partition_id_tensor.name if nc.partition_id_tensor else None
    in_names, out_names, out_avals, zero_outs = [], [], [], []
    for alloc in nc.m.functions[0].allocations:
        if not isinstance(alloc, _mb.MemoryLocationSet):
            continue
        name = alloc.memorylocations[0].name
        if alloc.kind == "ExternalInput":
            if name != partition_name:
                in_names.append(name)
        elif alloc.kind == "ExternalOutput":
            out_names.append(name)
            shape = tuple(alloc.tensor_shape)
            dtype = _mb.dt.np(alloc.dtype)
            out_avals.append(jax.core.ShapedArray(shape, dtype))
            zero_outs.append(np.zeros(shape, dtype))
    n_params = len(in_names)
    n_outs = len(out_avals)
    all_in_names = list(in_names) + list(out_names)
    if partition_name is not None:
        all_in_names.append(partition_name)

    def _bind(ins, outs):
        operands = list(ins) + list(outs)
        if partition_name is not None:
            operands.append(bass2jax.partition_id_tensor())
        return bass2jax._bass_exec_p.bind(
            *operands,
            out_avals=tuple(out_avals),
            in_names=tuple(all_in_names),
            out_names=tuple(out_names),
            lowering_input_output_aliases=(),
            sim_require_finite=True,
            sim_require_nnan=True,
            nc=nc,
        )

    def _body(*args):
        return tuple(_bind(args[:n_params], args[n_params:]))

    devices = jax.devices()[:NCORES]
    mesh = Mesh(np.asarray(devices), ("core",))
    in_specs = (PartitionSpec("core"),) * (n_params + n_outs)
    out_specs = (PartitionSpec("core"),) * len(out_names)
    sharded = jax.jit(
        shard_map(
            _body, mesh=mesh, in_specs=in_specs, out_specs=out_specs, check_rep=False
        ),
        keep_unused=True,
    )

    def prepare(in_maps):
        from jax.sharding import NamedSharding

        sh = NamedSharding(mesh, PartitionSpec("core"))
        concat_in = [
            jax.device_put(
                np.concatenate([np.asarray(m[name]) for m in in_maps], axis=0), sh
            )
            for name in in_names
        ]
        concat_zeros = [
            jax.device_put(np.zeros((NCORES * z.shape[0], *z.shape[1:]), z.dtype), sh)
            for z in zero_outs
        ]
        return concat_in, concat_zeros

    def execute(args):
        concat_in, concat_zeros = args
        return jax.block_until_ready(sharded(*concat_in, *concat_zeros))

    def run(in_maps, timing_reps=0):
        import time as _time

        concat_in, concat_zeros = prepare(in_maps)
        out_arrs = jax.block_until_ready(sharded(*concat_in, *concat_zeros))
        times = []
        if timing_reps:
            for _ in range(timing_reps):
                t0 = _time.perf_counter()
                jax.block_until_ready(sharded(*concat_in, *concat_zeros))
                times.append(_time.perf_counter() - t0)
        results = [
            {
                name: np.asarray(out_arrs[i]).reshape(NCORES, *out_avals[i].shape)[c]
                for i, name in enumerate(out_names)
            }
            for c in range(NCORES)
        ]
        return results, times

    run.prepare = prepare
    run.execute = execute
    return run


def _get_runner(repeat=1):
    key = f"runner{repeat}"
    if key not in _CACHE:
        _CACHE[key] = _make_runner(_wrap_to_json(_build_nc(repeat=repeat)))
    return _CACHE[key]


def _run(inputs, trace=False, timing_reps=0):
    in_maps = _prep_inputs(inputs)
    results, times = _get_runner(1)(in_maps, timing_reps=timing_reps)
    y = np.empty((B, T, F), np.float32)
    for c in range(NCORES):
        yt = results[c]["yT"].astype(np.float32)  # [T, 64, FD]
        y[c * Bc : c * Bc + FD] = yt[:, 0:24, :].transpose(2, 0, 1)
        y[c * Bc + FD : (c + 1) * Bc] = yt[:, 32:56, :].transpose(2, 0, 1)
    bd = np.asarray(inputs["bd"], np.float32)
    if bd.any():
        y += bd  # dense bias is linear; applied host-side
    return y, times


def kernel(**inputs):
    y, _ = _run(inputs)
    return y


# revision 50
# speedup vs baseline: 1.0025x; 1.0025x over previous
"""LSTM autoencoder (B=8192, T=50, F=24; H1=64, LAT=32, H3=64) on 8 trn2 cores.

v15: packed-halves, fold-free layout. The partition dim carries
feature x batch-half (64 feats x 2 halves = 128 partitions, FD=512
columns) so every elementwise op runs full width and no cross-partition
fold copies are needed. Each LSTM1 gate gets its own PSUM bank filled
by two accumulating matmuls: a packed x-side MM and a block-diagonal
h-side MM; relu(g) folds into the scalar_tensor_tensor that forms
i*relu(g) (sigmoid/relu elision: c >= 0 so h = o*relu(c) = o*c).
LSTM2 interleaves with LSTM1 at matmul-lag 2 / elementwise-lag 3 (its
h-side MMs are emitted after the elementwise block that writes h2 --
emitting them earlier reads a stale H2). Phase B (decoder LSTM3 +
dense) runs two independent FD/2 column streams to hide the recurrence
chain. All weights ship in one packed [128, 21*128] DMA. The dense
bias is applied host-side (it is outside all nonlinearities).
LSTM2's elementwise emission order (SFI2, IG2, FC2, C2add before SO2,
H2mul) matters: the LSTM2 h2-loop is the period-setter in phase A and
this ordering minimizes its schedule gaps.
Measured: 346 us HW (baseline v7: 539 us same-day), rel_err 3.9e-3.
"""

import os
import sys

import numpy as np

sys.path.insert(0, "/opt/trn_rl_repo")

import concourse.bass as bass
import concourse.mybir as mybir
from concourse.tile import TileContext
from contextlib import ExitStack

B, T, F = 8192, 50, 24
H1, LAT, H3 = 64, 32, 64
NCORES = 8
Bc = B // NCORES  # 1024
FD = Bc // 2  # 512 free-dim columns per packed op

f16 = mybir.dt.float16
f32 = mybir.dt.float32
AF = mybir.ActivationFunctionType
Alu = mybir.AluOpType

_CACHE = {}

XROWS = 96  # x tile rows: [x_h0(0:24); 1(24); pad; x_h1(64:88); 1(88); pad]

# ---------------------------------------------------------------------------
# Toolchain compat: the walrus build in this container predates two features
# the current Tile framework emits: replace the EVSEM tail barrier with the
# legacy pseudo barrier, and split >1 sem waits per instruction into
# single-wait NoOps.
# ---------------------------------------------------------------------------

bass.Bass.all_engine_barrier = (
    lambda self, *, sem_only=False: self._nrt_pseudo_barrier()
)
bass.Bass.multi_engine_barrier = lambda self, engines: self._nrt_pseudo_barrier()


def _split_multi_waits(js: bytes) -> bytes:
    import json

    m = json.loads(js)
    for fn in m["functions"]:
        for blk in fn["blocks"]:
            out = []
            for inst in blk["instructions"]:
                si = inst.get("sync_info")
                waits = (si or {}).get("on_wait") or []
                if len(waits) > 1:
                    for k, w in enumerate(waits[:-1]):
                        out.append(
                            {
                                "name": f"{inst['name']}_w{k}",
                                "engine": inst["engine"],
                                "opcode": "NoOp",
                                "debug": inst.get("debug", 0),
                                "ins": [],
                                "outs": [],
                                "sync_info": {"on_update": [], "on_wait": [w]},
                            }
                        )
                    si["on_wait"] = [waits[-1]]
                out.append(inst)
            blk["instructions"] = out
    return json.dumps(m).encode()


def _wrap_to_json(nc):
    orig = nc.to_json_bytes
    nc.to_json_bytes = lambda: _split_multi_waits(orig())
    return nc


def _build_nc(repeat=1):
    nc = bass.Bass()

    xT_d = nc.dram_tensor("xT", [T, XROWS, FD], f16, kind="ExternalInput")
    # All weights packed into one [128, 21*128] f16 tensor (one DMA):
    # col blocks: wx1[4] | uh1[4] | wx2[2] | uh2[2] | wz3[4] | uh3[4] | wd
    WCOLS = 21 * 128
    wall_d = nc.dram_tensor("wall", [128, WCOLS], f16, kind="ExternalInput")
    yT_d = nc.dram_tensor("yT", [T, 64, FD], f16, kind="ExternalOutput")

    with TileContext(nc) as tc:
     for _rep in range(repeat):
      with ExitStack() as ctx:
        wp = ctx.enter_context(tc.tile_pool(name=f"wp{_rep}", bufs=1))
        st = ctx.enter_context(tc.tile_pool(name=f"st{_rep}", bufs=1))
        sp = ctx.enter_context(tc.tile_pool(name=f"sp{_rep}", bufs=2))
        op = ctx.enter_context(tc.tile_pool(name=f"op{_rep}", bufs=3))

        wall = wp.tile([128, WCOLS], f16, name="wall")
        nc.sync.dma_start(out=wall, in_=wall_d[:])
        def wslice(i, rows):
            return wall[0:rows, 128 * i : 128 * i + 128]
        wx1 = [wslice(g, XROWS) for g in range(4)]
        uh1 = [wslice(4 + g, 128) for g in range(4)]
        wx2 = [wslice(8 + g, 128) for g in range(2)]
        uh2 = [wslice(10 + g, 65) for g in range(2)]
        wz3 = [wslice(12 + g, 65) for g in range(4)]
        uh3 = [wslice(16 + g, 128) for g in range(4)]
        wd = wall[0:128, 20 * 128 : 20 * 128 + 64]

        # ---- state ---------------------------------------------------------
        xr = [st.tile([XROWS, FD], f16, name=f"X{i}") for i in range(4)]
        h1r = [st.tile([128, FD], f16, name=f"H1_{i}") for i in range(4)]
        C1 = st.tile([128, FD], f16, name="C1")
        H2 = st.tile([65, FD], f16, name="H2")  # row 64 = 1.0 (bias row)
        C2 = st.tile([64, FD], f16, name="C2")

        nc.vector.memset(h1r[3], 0)  # h1_{-1}
        nc.vector.memset(C1, 0)
        nc.vector.memset(H2[0:64, :], 0)
        nc.vector.memset(H2[64:65, :], 1.0)
        nc.vector.memset(C2, 0)

        nc.sync.dma_start(out=xr[0], in_=xT_d[0])
        nc.sync.dma_start(out=xr[1], in_=xT_d[1])

        # ---- phase A: LSTM1 (t=k) + lagged LSTM2 ---------------------------
        # Weight-array gate order: 0=f, 1=i, 2=g, 3=o.
        # LSTM2's matmuls lag 2 steps (t2m) but its sigmoid+elementwise lag 3
        # (t2e): the elementwise block then sits AFTER the current LSTM1
        # chain in every engine's priority order, so the scheduler can never
        # pop an LSTM2 tail op ahead of a chain op, and SL2's ready time
        # falls into ACT's idle window after SFO instead of racing SI.
        # psL2 is double-buffered (a/b) so the two in-flight LSTM2 steps
        # don't serialize on the bank.
        with tc.tile_pool(name=f"ppA{_rep}", bufs=1, space="PSUM") as ppa:
          psL2s = {}
          for k in range(T + 3):
            if k + 2 < T:
                nc.sync.dma_start(out=xr[(k + 2) % 4], in_=xT_d[k + 2])

            t2m = k - 2  # LSTM2 matmul step
            t2 = k - 3  # LSTM2 elementwise step
            if k < T:
                X = xr[k % 4]
                Hp = h1r[(k + 3) % 4]  # h1_{k-1}
                psI = ppa.tile([128, FD], f32, tag="psI")
                psFO = ppa.tile([128, 2 * FD], f32, tag="psFO")
                psG = ppa.tile([128, FD], f32, tag="psG")
                # x-MMs for I/F/O run during the pre-h-MM stall; the three
                # chain-critical h-MMs follow immediately after h1_{k-1}
                # lands; G (consumed later, by IG1) and LSTM2 trail.
                nc.tensor.matmul(psI, wx1[1], X, start=True, stop=False)
                nc.tensor.matmul(
                    psFO[:, 0:FD], wx1[0], X, start=True, stop=False
                )
                nc.tensor.matmul(
                    psFO[:, FD : 2 * FD], wx1[3], X, start=True, stop=False
                )
                nc.tensor.matmul(psI, uh1[1], Hp, start=False, stop=True)
                nc.tensor.matmul(psFO[:, 0:FD], uh1[0], Hp, start=False, stop=True)
                nc.tensor.matmul(
                    psFO[:, FD : 2 * FD], uh1[3], Hp, start=False, stop=True
                )
                nc.tensor.matmul(psG, wx1[2], X, start=True, stop=False)
                nc.tensor.matmul(psG, uh1[2], Hp, start=False, stop=True)

            if 0 <= t2m < T:
                psL2 = ppa.tile(
                    [128, 2 * FD], f32, tag=f"psL2{t2m % 2}", name=f"psL2{t2m % 2}"
                )
                psL2s[t2m] = psL2
                nc.tensor.matmul(
                    psL2[:, 0:FD], wx2[0], h1r[t2m % 4], start=True, stop=False
                )
                nc.tensor.matmul(
                    psL2[:, FD : 2 * FD], wx2[1], h1r[t2m % 4],
                    start=True, stop=False,
                )

            if k < T:
                SI = sp.tile([128, FD], f16, tag="SI")
                nc.scalar.activation(SI, psI, AF.Sigmoid)
                SIg = SI
                SF = sp.tile([128, FD], f16, tag="SF")
                nc.scalar.activation(SF, psFO[:, 0:FD], AF.Sigmoid)
                SO = sp.tile([128, FD], f16, tag="SO")
                nc.scalar.activation(SO, psFO[:, FD : 2 * FD], AF.Sigmoid)
                SOg = SO

                IG1 = sp.tile([128, FD], f16, tag="IG1")
                nc.vector.scalar_tensor_tensor(
                    IG1, psG, 0.0, SI, Alu.max, Alu.mult
                )  # relu(g1) * si1
                FC1 = sp.tile([128, FD], f16, tag="FC1")
                nc.vector.tensor_mul(FC1, SF, C1)
                nc.vector.tensor_add(C1, FC1, IG1)
                nc.vector.tensor_mul(h1r[k % 4], SO, C1)

            if 0 <= t2 < T:
                psL2 = psL2s.pop(t2)
                SFI2 = sp.tile([128, FD], f16, tag="SFI2")
                nc.scalar.activation(SFI2, psL2[:, 0:FD], AF.Sigmoid)
                FC2 = sp.tile([64, FD], f16, tag="FC2")
                nc.vector.tensor_mul(FC2, SFI2[0:64, :], C2)
                IG2 = sp.tile([64, FD], f16, tag="IG2")
                nc.vector.scalar_tensor_tensor(
                    IG2, psL2[64:128, FD : 2 * FD], 0.0, SFI2[64:128, :],
                    Alu.max, Alu.mult,
                )  # relu(g2) * si2
                nc.vector.tensor_add(C2, FC2, IG2)
                SO2 = sp.tile([64, FD], f16, tag="SO2")
                nc.scalar.activation(SO2, psL2[0:64, FD : 2 * FD], AF.Sigmoid)
                nc.vector.tensor_mul(H2[0:64, :], SO2, C2)

            if 0 <= t2m < T:
                # h-side MMs AFTER the elementwise block: they contract
                # h2_{t2m-1}, which the H2mul just above (step t2 = t2m-1)
                # writes. Emitting them earlier would read a stale H2.
                psL2 = psL2s[t2m]
                nc.tensor.matmul(psL2[:, 0:FD], uh2[0], H2, start=False, stop=True)
                nc.tensor.matmul(
                    psL2[:, FD : 2 * FD], uh2[1], H2, start=False, stop=True
                )

        # ---- phase B: LSTM3 + dense, two FD/2 column streams ---------------
        HF = FD // 2  # 256
        css = (slice(0, HF), slice(HF, FD))
        H3s = [st.tile([128, HF], f16, name=f"H3_{s}") for s in range(2)]
        C3s = [st.tile([128, HF], f16, name=f"C3_{s}") for s in range(2)]
        for s in range(2):
            nc.vector.memset(H3s[s], 0)
            nc.vector.memset(C3s[s], 0)

        with tc.tile_pool(name=f"ppB{_rep}", bufs=1, space="PSUM") as ppb:
          for t in range(T):
            psI3 = [None, None]
            psFO3 = [None, None]
            psG3 = [None, None]
            psD3 = [None, None]
            # z-side MMs (only need H2 + a free bank) lead the PE FIFO
            for s in range(2):
                psI3[s] = ppb.tile([128, HF], f32, tag=f"psI3{s}", name=f"psI3{s}")
                psFO3[s] = ppb.tile(
                    [128, 2 * HF], f32, tag=f"psFO3{s}", name=f"psFO3{s}"
                )
                psG3[s] = ppb.tile([128, HF], f32, tag=f"psG3{s}", name=f"psG3{s}")
                Z = H2[:, css[s]]
                nc.tensor.matmul(psI3[s], wz3[1][:, :], Z, start=True, stop=False)
                nc.tensor.matmul(
                    psFO3[s][:, 0:HF], wz3[0][:, :], Z, start=True, stop=False
                )
                nc.tensor.matmul(
                    psFO3[s][:, HF : 2 * HF], wz3[3][:, :], Z,
                    start=True, stop=False,
                )
                nc.tensor.matmul(psG3[s], wz3[2][:, :], Z, start=True, stop=False)
            for s in range(2):
                nc.tensor.matmul(psI3[s], uh3[1], H3s[s], start=False, stop=True)
                nc.tensor.matmul(
                    psFO3[s][:, 0:HF], uh3[0], H3s[s], start=False, stop=True
                )
                nc.tensor.matmul(
                    psFO3[s][:, HF : 2 * HF], uh3[3], H3s[s],
                    start=False, stop=True,
                )
                nc.tensor.matmul(psG3[s], uh3[2], H3s[s], start=False, stop=True)

            SI3s, SFO3s = [], []
            for s in range(2):
                SI3 = sp.tile([128, HF], f16, tag=f"SI3{s}", name=f"SI3{s}")
                nc.scalar.activation(SI3, psI3[s], AF.Sigmoid)
                SFO3 = sp.tile(
                    [128, 2 * HF], f16, tag=f"SFO3{s}", name=f"SFO3{s}"
                )
                nc.scalar.activation(SFO3, psFO3[s], AF.Sigmoid)
                SI3s.append(SI3)
                SFO3s.append(SFO3)

            for s in range(2):
                IG3 = sp.tile([128, HF], f16, tag=f"IG3{s}", name=f"IG3{s}")
                nc.vector.scalar_tensor_tensor(
                    IG3, psG3[s], 0.0, SI3s[s], Alu.max, Alu.mult
                )
                FC3 = sp.tile([128, HF], f16, tag=f"FC3{s}", name=f"FC3{s}")
                nc.vector.tensor_mul(FC3, SFO3s[s][:, 0:HF], C3s[s])
                nc.vector.tensor_add(C3s[s], FC3, IG3)
                nc.vector.tensor_mul(
                    H3s[s], SFO3s[s][:, HF : 2 * HF], C3s[s]
                )  # h3_t

            for s in range(2):
                psD3[s] = ppb.tile([64, HF], f32, tag=f"psD{s}", name=f"psD{s}")
                nc.tensor.matmul(psD3[s], wd, H3s[s], start=True, stop=True)
            for s in range(2):
                yt = op.tile([64, HF], f16, tag=f"yt{s}", name=f"yt{s}")
                nc.vector.tensor_copy(yt, psD3[s])  # bd added host-side
                nc.sync.dma_start(out=yT_d[t][:, css[s]], in_=yt)

    return nc


def _prep_inputs(inputs):
    """Host-side: shard batch, pack weights/x into the packed-halves layout."""
    x = np.asarray(inputs["x"], np.float32)
    W1, U1, b1 = (np.asarray(inputs[k], np.float32) for k in ("W1", "U1", "b1"))
    W2, U2, b2 = (np.asarray(inputs[k], np.float32) for k in ("W2", "U2", "b2"))
    W3, U3, b3 = (np.asarray(inputs[k], np.float32) for k in ("W3", "U3", "b3"))
    Wd, bd = (np.asarray(inputs[k], np.float32) for k in ("Wd", "bd"))

    # Reference gate column order is (i, f, g, o), each H wide.
    def gcols(Wm, H, g):
        idx = {"i": 0, "f": 1, "g": 2, "o": 3}[g]
        return Wm[:, idx * H : (idx + 1) * H]

    GATES = ("f", "i", "g", "o")

    # LSTM1: x-side [XROWS,128] (rows 0:24 x_h0, 24 bias, 64:88 x_h1, 88 bias)
    # h-side blockdiag [128,128]
    wx1 = np.zeros((4, XROWS, 128), np.float32)
    uh1 = np.zeros((4, 128, 128), np.float32)
    for gi, g in enumerate(GATES):
        Wg, Ug, bg = gcols(W1, H1, g), gcols(U1, H1, g), gcols(b1[None], H1, g)[0]
        wx1[gi, 0:24, 0:64] = Wg
        wx1[gi, 24, 0:64] = bg
        wx1[gi, 64:88, 64:128] = Wg
        wx1[gi, 88, 64:128] = bg
        uh1[gi, 0:64, 0:64] = Ug
        uh1[gi, 64:128, 64:128] = Ug

    # LSTM2 gate pairs (f,i) and (o,g): bank cols = [gA_h0, gA_h1, gB_h0, gB_h1]
    wx2 = np.zeros((2, 128, 128), np.float32)
    uh2 = np.zeros((2, 65, 128), np.float32)
    for pi, (ga, gb) in enumerate((("f", "i"), ("o", "g"))):
        for half in range(2):
            r0, r1 = half * 64, half * 64 + 64  # h1 rows for this half
            s0, s1 = half * 32, half * 32 + 32  # h2 rows for this half
            wx2[pi, r0:r1, s0 : s0 + 32] = gcols(W2, LAT, ga)
            wx2[pi, r0:r1, 64 + s0 : 64 + s0 + 32] = gcols(W2, LAT, gb)
            uh2[pi, s0:s1, s0 : s0 + 32] = gcols(U2, LAT, ga)
            uh2[pi, s0:s1, 64 + s0 : 64 + s0 + 32] = gcols(U2, LAT, gb)
            uh2[pi, 64, s0 : s0 + 32] = gcols(b2[None], LAT, ga)[0]
            uh2[pi, 64, 64 + s0 : 64 + s0 + 32] = gcols(b2[None], LAT, gb)[0]

    # LSTM3: z-side [65,128] (z rows 0:32 h0, 32:64 h1, 64 bias), h-side [128,128]
    wz3 = np.zeros((4, 65, 128), np.float32)
    uh3 = np.zeros((4, 128, 128), np.float32)
    for gi, g in enumerate(GATES):
        Wg, Ug, bg = gcols(W3, H3, g), gcols(U3, H3, g), gcols(b3[None], H3, g)[0]
        wz3[gi, 0:32, 0:64] = Wg
        wz3[gi, 32:64, 64:128] = Wg
        wz3[gi, 64, 0:64] = bg
        wz3[gi, 64, 64:128] = bg
        uh3[gi, 0:64, 0:64] = Ug
        uh3[gi, 64:128, 64:128] = Ug

    wd = np.zeros((128, 64), np.float32)
    wd[0:64, 0:24] = Wd
    wd[64:128, 32:56] = Wd
    wall = np.zeros((128, 21 * 128), np.float32)
    for g in range(4):
        wall[0:XROWS, 128 * g : 128 * g + 128] = wx1[g]
        wall[:, 128 * (4 + g) : 128 * (4 + g) + 128] = uh1[g]
        wall[0:65, 128 * (12 + g) : 128 * (12 + g) + 128] = wz3[g]
        wall[:, 128 * (16 + g) : 128 * (16 + g) + 128] = uh3[g]
    for g in range(2):
        wall[:, 128 * (8 + g) : 128 * (8 + g) + 128] = wx2[g]
        wall[0:65, 128 * (10 + g) : 128 * (10 + g) + 128] = uh2[g]
    wall[:, 20 * 128 : 20 * 128 + 64] = wd
    w16 = {"wall": wall.astype(np.float16)}

    in_maps = []
    for c in range(NCORES):
        xc = x[c * Bc : (c + 1) * Bc]  # [Bc, T, F]
        xt = np.zeros((T, XROWS, FD), np.float16)
        xt[:, 0:24, :] = xc[0:FD].transpose(1, 2, 0)
        xt[:, 24, :] = 1.0
        xt[:, 64:88, :] = xc[FD:Bc].transpose(1, 2, 0)
        xt[:, 88, :] = 1.0
        m = {"xT": np.ascontiguousarray(xt)}
        m.update(w16)
        in_maps.append(m)
    return in_maps


def _make_runner(nc):
    """Compile nc once into a sharded 8-core jit; returns run(in_maps)->results."""
    import jax
    from jax.sharding import Mesh, PartitionSpec
    from jax.experimental.shard_map import shard_map
    from concourse import bass2jax, mybir as _mb

    bass2jax.install_neuronx_cc_hook()

    partition_name = nc.partition_id_tensor.name if nc.partition_id_tensor else None
    in_names, out_names, out_avals, zero_outs = [], [], [], []
    for alloc in nc.m.functions[0].allocations:
        if not isinstance(alloc, _mb.MemoryLocationSet):
            continue
        name = alloc.memorylocations[0].name
        if alloc.kind == "ExternalInput":
            if name != partition_name:
                in_names.append(name)
        elif alloc.kind == "ExternalOutput":
            out_names.append(name)
            shape = tuple(alloc.tensor_shape)
            dtype = _mb.dt.np(alloc.dtype)
            out_avals.append(jax.core.ShapedArray(shape, dtype))
            zero_outs.append(np.zeros(shape, dtype))
    n_params = len(in_names)
    n_outs = len(out_avals)
    all_in_names = list(in_names) + list(out_names)
    if partition_name is not None:
        all_in_names.append(partition_name)

    def _bind(ins, outs):
        operands = list(ins) + list(outs)
        if partition_name is not None:
            operands.append(bass2jax.partition_id_tensor())
        return bass2jax._bass_exec_p.bind(
            *operands,
            out_avals=tuple(out_avals),
            in_names=tuple(all_in_names),
            out_names=tuple(out_names),
            lowering_input_output_aliases=(),
            sim_require_finite=True,
            sim_require_nnan=True,
            nc=nc,
        )

    def _body(*args):
        return tuple(_bind(args[:n_params], args[n_params:]))

    devices = jax.devices()[:NCORES]
    mesh = Mesh(np.asarray(devices), ("core",))
    in_specs = (PartitionSpec("core"),) * (n_params + n_outs)
    out_specs = (PartitionSpec("core"),) * len(out_names)
    sharded = jax.jit(
        shard_map(
            _body, mesh=mesh, in_specs=in_specs, out_specs=out_specs, check_rep=False
        ),
        keep_unused=True,
    )

    def prepare(in_maps):
        from jax.sharding import NamedSharding

        sh = NamedSharding(mesh, PartitionSpec("core"))
        concat_in = [
            jax.device_put(
                np.concatenate([np.asarray(m[name]) for m in in_maps], axis=0), sh
            )
            for name in in_names
        ]
        concat_zeros = [
            jax.device_put(np.zeros((NCORES * z.shape[0], *z.shape[1:]), z.dtype), sh)
            for z in zero_outs
        ]
        return concat_in, concat_zeros

    def execute(args):
        concat_in, concat_zeros = args
        return jax.block_until_ready(sharded(*concat_in, *concat_zeros))

    def run(in_maps, timing_reps=0):
        import time as _time

        concat_in, concat_zeros = prepare(in_maps)
        out_arrs = jax.block_until_ready(sharded(*concat_in, *concat_zeros))
        times = []
        if timing_reps:
            for _ in range(timing_reps):
                t0 = _time.perf_counter()
                jax.block_until_ready(sharded(*concat_in, *concat_zeros))
                times.append(_time.perf_counter() - t0)
        results = [
            {
                name: np.asarray(out_arrs[i]).reshape(NCORES, *out_avals[i].shape)[c]
                for i, name in enumerate(out_names)
            }
            for c in range(NCORES)
        ]
        return results, times

    run.prepare = prepare
    run.execute = execute
    return run


def _get_runner(repeat=1):
    key = f"runner{repeat}"
    if key not in _CACHE:
        _CACHE[key] = _make_runner(_wrap_to_json(_build_nc(repeat=repeat)))
    return _CACHE[key]


def _run(inputs, trace=False, timing_reps=0):
    in_maps = _prep_inputs(inputs)
    results, times = _get_runner(1)(in_maps, timing_reps=timing_reps)
    y = np.empty((B, T, F), np.float32)
    for c in range(NCORES):
        yt = results[c]["yT"].astype(np.float32)  # [T, 64, FD]
        y[c * Bc : c * Bc + FD] = yt[:, 0:24, :].transpose(2, 0, 1)
        y[c * Bc + FD : (c + 1) * Bc] = yt[:, 32:56, :].transpose(2, 0, 1)
    bd = np.asarray(inputs["bd"], np.float32)
    if bd.any():
        y += bd  # dense bias is linear; applied host-side
    return y, times


def kernel(**inputs):
    y, _ = _run(inputs)
    return y
